# revision 13
# baseline (speedup 1.0000x reference)
"""Trainium2 Bass kernel for nn_AttentionCircuit (moe_routing).

Sharding: 8 cores; core c owns batch c//4, token block c%4 (512 tokens).
The neuron-bank mix/restore and W_O are pointwise over tokens -> fully
data-parallel.  Attention needs full-sequence K/V, which are all-gathered
(one collective) within each 4-core group (one group per batch element).

Layout strategy: activations are kept feature-major ([d, t], token on the
matmul moving axis, free dim 512) so every matmul runs at full float32r
throughput.  Per-token mixture weights are broadcast to [128, T] tiles with
K=1 ones-matmuls on the TensorEngine.  The causal mask is added to the score
PSUM with an identity-matmul from a per-core mask table (additive -1600 ->
exp underflows to exactly 0), which keeps the SPMD graph identical on all
cores.
"""

from contextlib import ExitStack

import numpy as np

import concourse.bass as bass
import concourse.mybir as mybir
from concourse import bacc
from concourse.bass_utils import run_bass_kernel_spmd
from concourse.masks import make_identity
from concourse.tile import TileContext

B, S, D, R, H, N = 2, 2048, 1024, 128, 16, 16
DH = D // H  # 64
NCORES = 8
GROUP = 4          # cores per batch element
T = S // GROUP     # 512 tokens per core
NT = T // 128      # 4 token tiles of 128
ND = D // 128      # 8 feature chunks
KC = S // 128      # 16 k-chunks per batch
VW = H * (DH + 1)  # 1040: V_aug row width (per head: 64 values + 1 one)
KT_SZ = ND * 128 * T          # floats in the KT part of the kv buffer
V_SZ = NT * 128 * VW          # floats in the V_aug part
KV_SZ = KT_SZ + V_SZ
MASK_NEG = -1600.0            # additive pre-scale mask; exp(-1600/8) == 0.0

F32 = mybir.dt.float32
F32R = mybir.dt.float32r
EXP = mybir.ActivationFunctionType.Exp
MUL = mybir.AluOpType.mult


def build_kernel():
    nc = bacc.Bacc(num_devices=NCORES)

    xT = nc.declare_dram_parameter("xT", [D, T], F32, isOutput=False)
    w1 = nc.declare_dram_parameter("w1", [3, N, T], F32, isOutput=False)
    w2 = nc.declare_dram_parameter("w2", [3, N, T], F32, isOutput=False)
    f_qk = nc.declare_dram_parameter("f_qk", [N, D, R], F32, isOutput=False)
    f_v = nc.declare_dram_parameter("f_v", [N, D, R], F32, isOutput=False)
    r_qk = nc.declare_dram_parameter("r_qk", [N, R, D], F32, isOutput=False)
    r_v = nc.declare_dram_parameter("r_v", [N, R, D], F32, isOutput=False)
    w_ot = nc.declare_dram_parameter("w_ot", [D, D], F32, isOutput=False)
    masktab = nc.declare_dram_parameter("masktab", [KC, 128, T], F32, isOutput=False)
    out_e = nc.declare_dram_parameter("out", [D, T], F32, isOutput=True)

    kv_in = nc.dram_tensor("kv_in", [KV_SZ], F32)
    kv_out = nc.dram_tensor("kv_out", [GROUP, KV_SZ], F32)

    with TileContext(nc) as tc, ExitStack() as st:
        # Pools are stack-ordered (must release LIFO); push long-lived first.
        const = st.enter_context(tc.tile_pool(name="const", bufs=1))
        ps = st.enter_context(tc.tile_pool(name="ps", bufs=2, space="PSUM"))
        gv_cm = tc.tile_pool(name="gv", bufs=16)
        gvp = gv_cm.__enter__()
        gqk_cm = tc.tile_pool(name="gqk", bufs=16)
        gqk = gqk_cm.__enter__()
        pmw_cm = tc.tile_pool(name="pmw", bufs=2)
        pmw = pmw_cm.__enter__()
        p1_cm = tc.tile_pool(name="p1", bufs=1)
        p1 = p1_cm.__enter__()
        fs_cm = tc.tile_pool(name="fstream", bufs=2)
        fstream = fs_cm.__enter__()

        # --- constants / global loads ---
        ident = const.tile([128, 128], F32)
        make_identity(nc, ident[:, :])
        ident_r = const.tile([128, 128], F32R)
        nc.vector.tensor_copy(ident_r[:, :], ident[:, :])
        ones_f = const.tile([1, 128], F32)
        nc.vector.memset(ones_f[:, :], 1.0)
        ones_col = const.tile([1, 128], F32R)
        nc.vector.tensor_copy(ones_col[:, :], ones_f[:, :])
        qt_s = const.tile([128, ND, T], F32R)
        dn = const.tile([H, T], F32)
        dn_r = const.tile([H, T], F32)

        xt_s = p1.tile([128, ND, T], F32R)
        nc.sync.dma_start(
            out=xt_s[:, :, :],
            in_=xT.ap().bitcast(F32R).rearrange("(c p) t -> p c t", p=128),
        )
        hq = pmw.tile([128, T], F32, name="hq", bufs=1)
        hk = pmw.tile([128, T], F32, name="hk", bufs=1)
        hv = pmw.tile([128, T], F32, name="hv", bufs=1)

        def bcast_w(w_dram, a, n, tag):
            """Broadcast weight row w_dram[a, n, :] to a [128, T] SBUF tile.

            The row is DMA'd to a base-partition-0 staging tile (matmul
            requires both operands at the same base partition), then
            replicated across partitions with a K=1 ones-matmul.
            """
            row = pmw.tile([1, T], F32R, tag="wrow", name="row")
            nc.sync.dma_start(
                out=row[:, :], in_=w_dram.ap().bitcast(F32R)[a, n, :].unsqueeze(0)
            )
            p_bc = ps.tile([128, T], F32, tag="pd", name="p_bc")
            nc.tensor.matmul(
                p_bc[:, :], ones_col[:, :], row[:, :], start=True, stop=True
            )
            sb = pmw.tile([128, T], F32, tag=tag, name="sb_" + tag)
            nc.scalar.copy(sb[:, :], p_bc[:, :])
            return sb

        # ---- phase 1: t = x @ f[n] (transposed) and neuron mixing ----
        for n in range(N):
            fq_t = fstream.tile([128, ND, R], F32R, tag="fq")
            nc.sync.dma_start(
                out=fq_t[:, :, :],
                in_=f_qk.ap().bitcast(F32R)[n].rearrange("(c p) r -> p c r", p=128),
            )
            p_tqk = ps.tile([128, T], F32, tag="pa")
            for ci in range(ND):
                nc.tensor.matmul(
                    p_tqk[:, :],
                    fq_t[:, ci, :],
                    xt_s[:, ci, :],
                    start=(ci == 0),
                    stop=(ci == ND - 1),
                )
            wq_bc = bcast_w(w1, 0, n, "bq")
            wk_bc = bcast_w(w1, 1, n, "bk")
            if n == 0:
                nc.vector.tensor_mul(hq[:, :], p_tqk[:, :], wq_bc[:, :])
                nc.vector.tensor_mul(hk[:, :], p_tqk[:, :], wk_bc[:, :])
            else:
                t0 = pmw.tile([128, T], F32, tag="t0")
                nc.vector.tensor_mul(t0[:, :], p_tqk[:, :], wq_bc[:, :])
                nc.vector.tensor_add(hq[:, :], hq[:, :], t0[:, :])
                t1 = pmw.tile([128, T], F32, tag="t1")
                nc.vector.tensor_mul(t1[:, :], p_tqk[:, :], wk_bc[:, :])
                nc.vector.tensor_add(hk[:, :], hk[:, :], t1[:, :])

            fv_t = fstream.tile([128, ND, R], F32R, tag="fv")
            nc.sync.dma_start(
                out=fv_t[:, :, :],
                in_=f_v.ap().bitcast(F32R)[n].rearrange("(c p) r -> p c r", p=128),
            )
            p_tv = ps.tile([128, T], F32, tag="pb")
            for ci in range(ND):
                nc.tensor.matmul(
                    p_tv[:, :],
                    fv_t[:, ci, :],
                    xt_s[:, ci, :],
                    start=(ci == 0),
                    stop=(ci == ND - 1),
                )
            wv_bc = bcast_w(w1, 2, n, "bv")
            if n == 0:
                nc.vector.tensor_mul(hv[:, :], p_tv[:, :], wv_bc[:, :])
            else:
                t2 = pmw.tile([128, T], F32, tag="t2")
                nc.vector.tensor_mul(t2[:, :], p_tv[:, :], wv_bc[:, :])
                nc.vector.tensor_add(hv[:, :], hv[:, :], t2[:, :])

        fs_cm.__exit__(None, None, None)
        p1_cm.__exit__(None, None, None)

        # ---- phase 2a: g_m = h * w2[:, m] (feature-major) ----
        gq_t, gk_t, gv_t = [], [], []
        for m in range(N):
            wq2 = bcast_w(w2, 0, m, "bq")
            g0 = gqk.tile([128, T], F32R, tag="gq", name=f"gq{m}")
            nc.vector.tensor_mul(g0[:, :], hq[:, :], wq2[:, :])
            gq_t.append(g0)
            wk2 = bcast_w(w2, 1, m, "bk")
            g1 = gqk.tile([128, T], F32R, tag="gk", name=f"gk{m}")
            nc.vector.tensor_mul(g1[:, :], hk[:, :], wk2[:, :])
            gk_t.append(g1)
            wv2 = bcast_w(w2, 2, m, "bv")
            g2 = gvp.tile([128, T], F32R, tag="gv", name=f"gv{m}")
            nc.vector.tensor_mul(g2[:, :], hv[:, :], wv2[:, :])
            gv_t.append(g2)
        pmw_cm.__exit__(None, None, None)

        # ---- phase 2b: restore Q/K (share the r_qk stream), then V ----
        rs_cm = tc.tile_pool(name="rstream", bufs=4)
        rstream = rs_cm.__enter__()
        kvt_cm = tc.tile_pool(name="kvtmp", bufs=2)
        kvtmp = kvt_cm.__enter__()
        pv_cm = tc.tile_pool(name="pv", bufs=1)
        pvp = pv_cm.__enter__()

        for ci in range(ND):
            p_q = ps.tile([128, T], F32, tag="pa")
            p_k = ps.tile([128, T], F32, tag="pb")
            for m in range(N):
                rt = rstream.tile([128, 128], F32R, tag="rt")
                nc.sync.dma_start(
                    out=rt[:, :],
                    in_=r_qk.ap().bitcast(F32R)[m, :, ci * 128 : (ci + 1) * 128],
                )
                nc.tensor.matmul(
                    p_q[:, :], rt[:, :], gq_t[m][:, :],
                    start=(m == 0), stop=(m == N - 1),
                )
                nc.tensor.matmul(
                    p_k[:, :], rt[:, :], gk_t[m][:, :],
                    start=(m == 0), stop=(m == N - 1),
                )
            nc.vector.tensor_copy(qt_s[:, ci, :], p_q[:, :])
            kt_tmp = kvtmp.tile([128, T], F32, tag="kt")
            nc.vector.tensor_copy(kt_tmp[:, :], p_k[:, :])
            nc.sync.dma_start(
                out=kv_in.ap()[ci * 128 * T : (ci + 1) * 128 * T].rearrange(
                    "(p t) -> p t", p=128
                ),
                in_=kt_tmp[:, :],
            )

        vaug_t = pvp.tile([128, NT, VW], F32)
        for tt in range(NT):
            nc.vector.memset(
                vaug_t[:, tt, :].rearrange("p (h e) -> p h e", e=DH + 1)[:, :, DH : DH + 1],
                1.0,
            )
        for ci in range(ND):
            p_v = ps.tile([128, T], F32, tag="pa")
            for m in range(N):
                rt = rstream.tile([128, 128], F32R, tag="rt")
                nc.sync.dma_start(
                    out=rt[:, :],
                    in_=r_v.ap().bitcast(F32R)[m, :, ci * 128 : (ci + 1) * 128],
                )
                nc.tensor.matmul(
                    p_v[:, :], rt[:, :], gv_t[m][:, :],
                    start=(m == 0), stop=(m == N - 1),
                )
            vt_tmp = kvtmp.tile([128, T], F32, tag="vt")
            nc.vector.tensor_copy(vt_tmp[:, :], p_v[:, :])
            for tt in range(NT):
                tp = ps.tile([128, 128], F32, tag="pc")
                nc.tensor.transpose(
                    tp[:, :], vt_tmp[:, tt * 128 : (tt + 1) * 128], ident[:, :]
                )
                dst = vaug_t[:, tt, :].rearrange("p (h e) -> p h e", e=DH + 1)[
                    :, 2 * ci : 2 * ci + 2, 0:DH
                ]
                nc.vector.tensor_copy(
                    dst, tp[:, :].rearrange("p (h e) -> p h e", e=DH)
                )
        nc.sync.dma_start(
            out=kv_in.ap()[KT_SZ:].rearrange("(t p f) -> p t f", t=NT, p=128),
            in_=vaug_t[:, :, :],
        )
        pv_cm.__exit__(None, None, None)
        kvt_cm.__exit__(None, None, None)
        rs_cm.__exit__(None, None, None)
        gqk_cm.__exit__(None, None, None)
        gv_cm.__exit__(None, None, None)

        # ---- collective: all-gather K/V within each batch group ----
        nc.gpsimd.collective_compute(
            "AllGather",
            mybir.AluOpType.bypass,
            replica_groups=[[0, 1, 2, 3], [4, 5, 6, 7]],
            ins=[kv_in.ap().opt()],
            outs=[kv_out.ap().opt()],
        )

        # W_O weights: load while the collective is in flight
        wotp_cm = tc.tile_pool(name="wotp", bufs=1)
        wotp = wotp_cm.__enter__()
        otp_cm = tc.tile_pool(name="ot", bufs=16)
        otp = otp_cm.__enter__()
        wot_s = wotp.tile([128, ND, D], F32R)
        nc.sync.dma_start(
            out=wot_s[:, :, :],
            in_=w_ot.ap().bitcast(F32R).rearrange("(c p) e -> p c e", p=128),
        )

        # ---- phase 3: attention (uniform masked full-causal) ----
        kvs_cm = tc.tile_pool(name="kvs", bufs=2)
        kvs = kvs_cm.__enter__()
        mts_cm = tc.tile_pool(name="mts", bufs=2)
        mts = mts_cm.__enter__()
        pts_cm = tc.tile_pool(name="pts", bufs=3)
        pts = pts_cm.__enter__()
        ot_acc = [otp.tile([DH + 1, T], F32, tag="ot", name=f"ot{h}") for h in range(H)]
        for ksb in range(GROUP):
            kt_b = kvs.tile([128, ND, T], F32R, tag="ktb")
            nc.sync.dma_start(
                out=kt_b[:, :, :],
                in_=kv_out.ap().bitcast(F32R)[ksb, :KT_SZ].rearrange(
                    "(c p t) -> p c t", c=ND, p=128
                ),
            )
            va_b = kvs.tile([128, NT, VW], F32R, tag="vab")
            nc.sync.dma_start(
                out=va_b[:, :, :],
                in_=kv_out.ap().bitcast(F32R)[ksb, KT_SZ:].rearrange(
                    "(t p f) -> p t f", t=NT, p=128
                ),
            )
            mt = mts.tile([128, NT, T], F32R, tag="mt")
            nc.sync.dma_start(
                out=mt[:, :, :],
                in_=masktab.ap().bitcast(F32R)[ksb * NT : (ksb + 1) * NT].rearrange(
                    "k p t -> p k t"
                ),
            )
            for h in range(H):
                hp = (h % 2) * DH
                hc = h // 2
                p_o = ps.tile([DH + 1, T], F32, tag="pb")
                for kc4 in range(NT):
                    p_s = ps.tile([128, T], F32, tag="pa")
                    nc.tensor.matmul(
                        p_s[:, :], ident_r[:, :], mt[:, kc4, :],
                        start=True, stop=False,
                    )
                    nc.tensor.matmul(
                        p_s[:, :],
                        kt_b[hp : hp + DH, hc, kc4 * 128 : (kc4 + 1) * 128],
                        qt_s[hp : hp + DH, hc, :],
                        start=False, stop=True,
                    )
                    pt = pts.tile([128, T], F32R, tag="pt")
                    nc.scalar.activation(pt[:, :], p_s[:, :], EXP, scale=0.125)
                    nc.tensor.matmul(
                        p_o[:, :],
                        va_b[:, kc4, h * (DH + 1) : (h + 1) * (DH + 1)],
                        pt[:, :],
                        start=(kc4 == 0), stop=(kc4 == NT - 1),
                    )
                if ksb == 0:
                    nc.vector.tensor_copy(ot_acc[h][:, :], p_o[:, :])
                else:
                    nc.vector.tensor_add(ot_acc[h][:, :], ot_acc[h][:, :], p_o[:, :])
        pts_cm.__exit__(None, None, None)
        mts_cm.__exit__(None, None, None)
        kvs_cm.__exit__(None, None, None)

        # ---- normalize (denominator = ones-row of V_aug, in ot row DH) ----
        np4_cm = tc.tile_pool(name="np4", bufs=2)
        np4 = np4_cm.__enter__()
        for h in range(H):
            # cross-partition row move: use DMA (engines can't shift partitions)
            nc.sync.dma_start(out=dn[h : h + 1, :], in_=ot_acc[h][DH : DH + 1, :])
        nc.vector.reciprocal(dn_r[:, :], dn[:, :])
        attn_t = np4.tile([128, ND, T], F32R, bufs=1)
        for h in range(H):
            stg = np4.tile([1, T], F32R, tag="stg")
            nc.sync.dma_start(out=stg[:, :], in_=dn_r[h : h + 1, :].bitcast(F32R))
            p_nb = ps.tile([DH, T], F32, tag="pc")
            nc.tensor.matmul(
                p_nb[:, :], ones_col[0:1, 0:DH], stg[:, :],
                start=True, stop=True,
            )
            nc.vector.tensor_mul(
                ot_acc[h][0:DH, :], ot_acc[h][0:DH, :], p_nb[:, :]
            )
            nc.sync.dma_start(
                out=attn_t[(h % 2) * DH : (h % 2) * DH + DH, h // 2, :],
                in_=ot_acc[h][0:DH, :].bitcast(F32R),
            )

        # ---- phase 4: out.T = W_O @ attn.T ----
        for c2 in range(ND):
            p_out = ps.tile([128, T], F32, tag="pa")
            for c1 in range(ND):
                nc.tensor.matmul(
                    p_out[:, :],
                    wot_s[:, c1, c2 * 128 : (c2 + 1) * 128],
                    attn_t[:, c1, :],
                    start=(c1 == 0), stop=(c1 == ND - 1),
                )
            oo = np4.tile([128, T], F32, tag="oo")
            nc.vector.tensor_copy(oo[:, :], p_out[:, :])
            nc.sync.dma_start(
                out=out_e.ap()[c2 * 128 : (c2 + 1) * 128, :], in_=oo[:, :]
            )
        np4_cm.__exit__(None, None, None)
        otp_cm.__exit__(None, None, None)
        wotp_cm.__exit__(None, None, None)

    return nc


_NC_CACHE = None


def _get_nc():
    global _NC_CACHE
    if _NC_CACHE is None:
        nc = build_kernel()
        if not nc.is_finalized():
            nc.finalize()  # Bacc: runs wait legalization + reg alloc
        _NC_CACHE = nc
    return _NC_CACHE


def _make_masktab(g):
    """Additive pre-scale causal mask table for a core owning token block g."""
    tab = np.zeros((KC, 128, T), dtype=np.float32)
    for kc in range(KC):
        if kc < g * NT:
            continue
        if kc >= (g + 1) * NT:
            tab[kc, :, :] = MASK_NEG
        else:
            o = (kc - g * NT) * 128
            p = np.arange(128)[:, None]
            j = np.arange(T)[None, :]
            tab[kc] = np.where(o + p <= j, 0.0, MASK_NEG).astype(np.float32)
    return tab


def make_in_maps(inputs):
    x = np.ascontiguousarray(inputs["x"], dtype=np.float32)
    f_qk = np.ascontiguousarray(inputs["f_qk"], dtype=np.float32)
    f_v = np.ascontiguousarray(inputs["f_v"], dtype=np.float32)
    r_qk = np.ascontiguousarray(inputs["r_qk"], dtype=np.float32)
    r_v = np.ascontiguousarray(inputs["r_v"], dtype=np.float32)
    w_ot = np.ascontiguousarray(inputs["W_O"].T, dtype=np.float32)

    in_maps = []
    for c in range(NCORES):
        b, g = c // GROUP, c % GROUP
        sl = slice(g * T, (g + 1) * T)
        w1 = np.stack(
            [
                inputs["fqk_weights_Q"][b, sl].T,
                inputs["fqk_weights_K"][b, sl].T,
                inputs["fv_weights"][b, sl].T,
            ]
        ).astype(np.float32)
        w2 = np.stack(
            [
                inputs["rqk_weights_Q"][b, sl].T,
                inputs["rqk_weights_K"][b, sl].T,
                inputs["rv_weights"][b, sl].T,
            ]
        ).astype(np.float32)
        in_maps.append(
            {
                "xT": np.ascontiguousarray(x[b, sl].T),
                "w1": np.ascontiguousarray(w1),
                "w2": np.ascontiguousarray(w2),
                "f_qk": f_qk,
                "f_v": f_v,
                "r_qk": r_qk,
                "r_v": r_v,
                "w_ot": w_ot,
                "masktab": _make_masktab(g),
            }
        )
    return in_maps


def assemble_output(results):
    out = np.zeros((B, S, D), dtype=np.float32)
    for c in range(NCORES):
        b, g = c // GROUP, c % GROUP
        out[b, g * T : (g + 1) * T, :] = results[c]["out"].T
    return out


def kernel(**inputs):
    nc = _get_nc()
    in_maps = make_in_maps(inputs)
    res = run_bass_kernel_spmd(nc, in_maps, core_ids=list(range(NCORES)))
    return assemble_output(res.results)


if __name__ == "__main__":
    nc = build_kernel()
    print("built ok")


# revision 17
# speedup vs baseline: 1.4073x; 1.4073x over previous
"""Trainium2 Bass kernel for nn_AttentionCircuit (moe_routing).

Sharding: 8 cores; core c owns batch c//4, token block c%4 (512 tokens).
The neuron-bank mix/restore and W_O are pointwise over tokens -> fully
data-parallel.  Attention is head-sharded within each 4-core batch group:
an AllToAll exchanges (Q,K,V_aug) so each core holds 4 heads x the full
2048-token sequence, computes exactly the causal blocks it needs (same
static instruction stream on every core), and a second small AllToAll
brings the normalized attention rows back to token-sharding for W_O.

Layouts: activations feature-major ([d, t], token on the moving axis, free
dim 512) so every matmul runs at full float32r speed.  V/probability tiles
use bf16 (halves the big collective).  Per-token mixture weights are
broadcast to [128, T] tiles on the otherwise-idle GPSIMD engine.  The
causal mask is added pre-exp in PSUM via an identity-matmul with a small
diagonal mask table (additive -1600 -> exp underflows to exactly 0).
"""

from contextlib import ExitStack

import numpy as np

import concourse.bass as bass
import concourse.mybir as mybir
from concourse import bacc
from concourse.bass_utils import run_bass_kernel_spmd
from concourse.masks import make_identity
from concourse.tile import TileContext

B, S, D, R, H, N = 2, 2048, 1024, 128, 16, 16
DH = D // H  # 64
NCORES = 8
GROUP = 4          # cores per batch element
T = S // GROUP     # 512 tokens per core
NT = T // 128      # 4 token tiles of 128
ND = D // 128      # 8 feature chunks
LH = H // NCORES   # 2 local heads after the 8-way all-to-all
VWL = LH * (DH + 1)  # 130: V_aug row width per shard (2 heads x (64+1))
MASK_NEG = -1600.0   # additive pre-scale mask; exp(-1600/8) == 0.0 in fp32

# A2A#1 shard layout (floats): Q [128,T] | K [128,T] | V_aug bf16 [NT,128,VWL]
QK_SZ = 128 * T              # 65536 floats each for Q and K
V_SZB = NT * 128 * VWL       # 66560 bf16 elements
SH1 = 2 * QK_SZ + V_SZB // 2  # shard size in f32 words
VOFF = 2 * QK_SZ             # f32-word offset of V region

F32 = mybir.dt.float32
F32R = mybir.dt.float32r
BF16 = mybir.dt.bfloat16
EXP = mybir.ActivationFunctionType.Exp


def build_kernel():
    nc = bacc.Bacc(num_devices=NCORES)

    xT = nc.declare_dram_parameter("xT", [D, T], F32, isOutput=False)
    w1 = nc.declare_dram_parameter("w1", [3, N, T], F32, isOutput=False)
    w2 = nc.declare_dram_parameter("w2", [3, N, T], F32, isOutput=False)
    f_qk = nc.declare_dram_parameter("f_qk", [N, D, R], F32, isOutput=False)
    f_v = nc.declare_dram_parameter("f_v", [N, D, R], F32, isOutput=False)
    r_qk = nc.declare_dram_parameter("r_qk", [N, R, D], F32, isOutput=False)
    r_v = nc.declare_dram_parameter("r_v", [N, R, D], F32, isOutput=False)
    w_ot = nc.declare_dram_parameter("w_ot", [D, D], F32, isOutput=False)
    maskd = nc.declare_dram_parameter("maskd", [NT, 128, T], F32, isOutput=False)
    out_e = nc.declare_dram_parameter("out", [D, T], F32, isOutput=True)

    a1_in = nc.dram_tensor("a1_in", [NCORES, SH1], F32)
    a1_out = nc.dram_tensor("a1_out", [NCORES, SH1], F32)
    a2_in = nc.dram_tensor("a2_in", [NCORES, LH * DH, T], F32)
    a2_out = nc.dram_tensor("a2_out", [NCORES, LH * DH, T], F32)
    RG = [list(range(NCORES))]

    with TileContext(nc) as tc, ExitStack() as st:
        # Pools are stack-ordered (release LIFO); space reserved at creation.
        const = st.enter_context(tc.tile_pool(name="const", bufs=1))
        ps = st.enter_context(tc.tile_pool(name="ps", bufs=2, space="PSUM"))
        gv_cm = tc.tile_pool(name="gv", bufs=16)
        gvp = gv_cm.__enter__()
        gqk_cm = tc.tile_pool(name="gqk", bufs=16)
        gqk = gqk_cm.__enter__()
        pmw_cm = tc.tile_pool(name="pmw", bufs=2)
        pmw = pmw_cm.__enter__()
        p1_cm = tc.tile_pool(name="p1", bufs=1)
        p1 = p1_cm.__enter__()
        fs_cm = tc.tile_pool(name="fstream", bufs=2)
        fstream = fs_cm.__enter__()

        # --- constants ---
        ident = const.tile([128, 128], F32)
        make_identity(nc, ident[:, :])
        ident_r = const.tile([128, 128], F32R)
        nc.vector.tensor_copy(ident_r[:, :], ident[:, :])
        ones_f = const.tile([65, 128], F32)
        nc.vector.memset(ones_f[:, :], 1.0)
        ones65 = const.tile([65, 128], F32R)
        nc.vector.tensor_copy(ones65[:, :], ones_f[:, :])

        xt_s = p1.tile([128, ND, T], F32R)
        nc.sync.dma_start(
            out=xt_s[:, :, :],
            in_=xT.ap().bitcast(F32R).rearrange("(c p) t -> p c t", p=128),
        )
        hq = pmw.tile([128, T], F32, name="hq", bufs=1)
        hk = pmw.tile([128, T], F32, name="hk", bufs=1)
        hv = pmw.tile([128, T], F32, name="hv", bufs=1)

        def bcast_w(w_dram, a, n, tag):
            """Broadcast weight row w_dram[a, n, :] to a [128, T] tile on
            GPSIMD (the engine is otherwise idle; frees PE/ACT)."""
            row = pmw.tile([1, T], F32, tag="wrow", name="row")
            nc.sync.dma_start(out=row[:, :], in_=w_dram.ap()[a, n, :].unsqueeze(0))
            sb = pmw.tile([128, T], F32, tag=tag, name="sb_" + tag)
            nc.gpsimd.partition_broadcast(sb[:, :], row[:, :])
            return sb

        # ---- phase 1: tT[n] = (x @ f[n]).T and neuron mixing ----
        for n in range(N):
            fq_t = fstream.tile([128, ND, R], F32R, tag="fq")
            nc.sync.dma_start(
                out=fq_t[:, :, :],
                in_=f_qk.ap().bitcast(F32R)[n].rearrange("(c p) r -> p c r", p=128),
            )
            p_tqk = ps.tile([128, T], F32, tag="pa", bufs=3)
            for ci in range(ND):
                nc.tensor.matmul(
                    p_tqk[:, :], fq_t[:, ci, :], xt_s[:, ci, :],
                    start=(ci == 0), stop=(ci == ND - 1),
                )
            wq_bc = bcast_w(w1, 0, n, "bq")
            wk_bc = bcast_w(w1, 1, n, "bk")
            if n == 0:
                nc.vector.tensor_mul(hq[:, :], p_tqk[:, :], wq_bc[:, :])
                nc.vector.tensor_mul(hk[:, :], p_tqk[:, :], wk_bc[:, :])
            else:
                t0 = pmw.tile([128, T], F32, tag="t0")
                nc.vector.tensor_mul(t0[:, :], p_tqk[:, :], wq_bc[:, :])
                nc.vector.tensor_add(hq[:, :], hq[:, :], t0[:, :])
                t1 = pmw.tile([128, T], F32, tag="t1")
                nc.vector.tensor_mul(t1[:, :], p_tqk[:, :], wk_bc[:, :])
                nc.vector.tensor_add(hk[:, :], hk[:, :], t1[:, :])

            fv_t = fstream.tile([128, ND, R], F32R, tag="fv")
            nc.sync.dma_start(
                out=fv_t[:, :, :],
                in_=f_v.ap().bitcast(F32R)[n].rearrange("(c p) r -> p c r", p=128),
            )
            p_tv = ps.tile([128, T], F32, tag="pb")
            for ci in range(ND):
                nc.tensor.matmul(
                    p_tv[:, :], fv_t[:, ci, :], xt_s[:, ci, :],
                    start=(ci == 0), stop=(ci == ND - 1),
                )
            wv_bc = bcast_w(w1, 2, n, "bv")
            if n == 0:
                nc.vector.tensor_mul(hv[:, :], p_tv[:, :], wv_bc[:, :])
            else:
                t2 = pmw.tile([128, T], F32, tag="t2")
                nc.vector.tensor_mul(t2[:, :], p_tv[:, :], wv_bc[:, :])
                nc.vector.tensor_add(hv[:, :], hv[:, :], t2[:, :])

        fs_cm.__exit__(None, None, None)
        p1_cm.__exit__(None, None, None)

        # ---- phase 2a: g_m = h * w2[:, m] ----
        gq_t, gk_t, gv_t = [], [], []
        for m in range(N):
            wq2 = bcast_w(w2, 0, m, "bq")
            g0 = gqk.tile([128, T], F32R, tag="gq", name=f"gq{m}")
            nc.vector.tensor_mul(g0[:, :], hq[:, :], wq2[:, :])
            gq_t.append(g0)
            wk2 = bcast_w(w2, 1, m, "bk")
            g1 = gqk.tile([128, T], F32R, tag="gk", name=f"gk{m}")
            nc.vector.tensor_mul(g1[:, :], hk[:, :], wk2[:, :])
            gk_t.append(g1)
            wv2 = bcast_w(w2, 2, m, "bv")
            g2 = gvp.tile([128, T], F32R, tag="gv", name=f"gv{m}")
            nc.vector.tensor_mul(g2[:, :], hv[:, :], wv2[:, :])
            gv_t.append(g2)
        pmw_cm.__exit__(None, None, None)

        # ---- phase 2b: restore Q/K (share the r_qk stream), then V ----
        rs_cm = tc.tile_pool(name="rstream", bufs=4)
        rstream = rs_cm.__enter__()
        kvt_cm = tc.tile_pool(name="kvtmp", bufs=2)
        kvtmp = kvt_cm.__enter__()
        pv_cm = tc.tile_pool(name="pv", bufs=1)
        pvp = pv_cm.__enter__()

        for ci in range(ND):
            p_q = ps.tile([128, T], F32, tag="pa", bufs=3)
            p_k = ps.tile([128, T], F32, tag="pb")
            for m in range(N):
                rt = rstream.tile([128, 128], F32R, tag="rt")
                nc.sync.dma_start(
                    out=rt[:, :],
                    in_=r_qk.ap().bitcast(F32R)[m, :, ci * 128 : (ci + 1) * 128],
                )
                nc.tensor.matmul(
                    p_q[:, :], rt[:, :], gq_t[m][:, :],
                    start=(m == 0), stop=(m == N - 1),
                )
                nc.tensor.matmul(
                    p_k[:, :], rt[:, :], gk_t[m][:, :],
                    start=(m == 0), stop=(m == N - 1),
                )
            # export Q/K chunk ci to a2a shard ci (heads 2ci, 2ci+1)
            qt_tmp = kvtmp.tile([128, T], F32, tag="qt")
            nc.vector.tensor_copy(qt_tmp[:, :], p_q[:, :])
            nc.sync.dma_start(
                out=a1_in.ap()[ci, 0:QK_SZ].rearrange("(p t) -> p t", p=128),
                in_=qt_tmp[:, :],
            )
            kt_tmp = kvtmp.tile([128, T], F32, tag="kt")
            nc.vector.tensor_copy(kt_tmp[:, :], p_k[:, :])
            nc.sync.dma_start(
                out=a1_in.ap()[ci, QK_SZ : 2 * QK_SZ].rearrange(
                    "(p t) -> p t", p=128
                ),
                in_=kt_tmp[:, :],
            )

        vaug_b = pvp.tile([128, NT, NCORES * VWL], BF16)
        for tt in range(NT):
            nc.vector.memset(
                vaug_b[:, tt, :].rearrange("p (h e) -> p h e", e=DH + 1)[
                    :, :, DH : DH + 1
                ],
                1.0,
            )
        for ci in range(ND):
            p_v = ps.tile([128, T], F32, tag="pa", bufs=3)
            for m in range(N):
                rt = rstream.tile([128, 128], F32R, tag="rt")
                nc.sync.dma_start(
                    out=rt[:, :],
                    in_=r_v.ap().bitcast(F32R)[m, :, ci * 128 : (ci + 1) * 128],
                )
                nc.tensor.matmul(
                    p_v[:, :], rt[:, :], gv_t[m][:, :],
                    start=(m == 0), stop=(m == N - 1),
                )
            vt_tmp = kvtmp.tile([128, T], F32, tag="vt")
            nc.vector.tensor_copy(vt_tmp[:, :], p_v[:, :])
            for tt in range(NT):
                tp = ps.tile([128, 128], F32, tag="pc")
                nc.tensor.transpose(
                    tp[:, :], vt_tmp[:, tt * 128 : (tt + 1) * 128], ident[:, :]
                )
                dst = vaug_b[:, tt, :].rearrange("p (h e) -> p h e", e=DH + 1)[
                    :, 2 * ci : 2 * ci + 2, 0:DH
                ]
                nc.vector.tensor_copy(
                    dst, tp[:, :].rearrange("p (h e) -> p h e", e=DH)
                )
        # export V_aug per shard (bf16 region of a1_in)
        a1_in_b = a1_in.ap().bitcast(BF16)  # [NCORES, 2*SH1] bf16
        for r in range(NCORES):
            nc.sync.dma_start(
                out=a1_in_b[r, 2 * VOFF : 2 * VOFF + V_SZB].rearrange(
                    "(t p f) -> p t f", t=NT, p=128
                ),
                in_=vaug_b[:, :, r * VWL : (r + 1) * VWL],
            )
        pv_cm.__exit__(None, None, None)
        kvt_cm.__exit__(None, None, None)
        rs_cm.__exit__(None, None, None)
        gqk_cm.__exit__(None, None, None)
        gv_cm.__exit__(None, None, None)

        # ---- A2A #1: exchange head-shards of Q/K/V within each batch group
        nc.gpsimd.collective_compute(
            "AllToAll",
            mybir.AluOpType.bypass,
            replica_groups=RG,
            ins=[a1_in.ap().opt()],
            outs=[a1_out.ap().opt()],
        )

        # loads that overlap the collective
        wot_cm = tc.tile_pool(name="wotp", bufs=1)
        wotp = wot_cm.__enter__()
        wot_s = wotp.tile([128, ND, D], F32R)
        nc.sync.dma_start(
            out=wot_s[:, :, :],
            in_=w_ot.ap().bitcast(F32R).rearrange("(c p) e -> p c e", p=128),
        )
        att_cm = tc.tile_pool(name="att", bufs=1)
        attp = att_cm.__enter__()
        maskt = attp.tile([128, NT, T], F32R)
        nc.sync.dma_start(
            out=maskt[:, :, :],
            in_=maskd.ap().bitcast(F32R).rearrange("k p t -> p k t"),
        )

        # gather local-head Q/K/V (both batches, full sequences)
        qt_l = attp.tile([128, NCORES * T], F32R)
        kt_l = attp.tile([128, NCORES * T], F32R)
        a1_out_r = a1_out.ap().bitcast(F32R)
        a1_out_b = a1_out.ap().bitcast(BF16)
        for j in range(NCORES):
            nc.sync.dma_start(
                out=qt_l[:, j * T : (j + 1) * T],
                in_=a1_out_r[j, 0:QK_SZ].rearrange("(p t) -> p t", p=128),
            )
            nc.sync.dma_start(
                out=kt_l[:, j * T : (j + 1) * T],
                in_=a1_out_r[j, QK_SZ : 2 * QK_SZ].rearrange("(p t) -> p t", p=128),
            )
        va_l = attp.tile([128, NCORES * NT, VWL], BF16)
        for j in range(NCORES):
            nc.sync.dma_start(
                out=va_l[:, j * NT : (j + 1) * NT, :],
                in_=a1_out_b[j, 2 * VOFF : 2 * VOFF + V_SZB].rearrange(
                    "(t p f) -> p t f", t=NT, p=128
                ),
            )

        # ---- phase 3: head-sharded causal attention ----
        pts_cm = tc.tile_pool(name="pts", bufs=4)
        pts = pts_cm.__enter__()
        ot_cm = tc.tile_pool(name="otst", bufs=3)
        otst = ot_cm.__enter__()

        def attend(bb, lh, qsb, p_o):
            """Scores+exp+PV for local head lh, batch bb, query block qsb
            (accumulates the whole causal k-row into one PSUM bank p_o)."""
            hp = lh * DH
            toff = bb * GROUP * T
            nkc = (qsb + 1) * NT
            for kc in range(nkc):
                p_s = ps.tile([128, T], F32, tag="pa", bufs=3, name="p_s")
                diag = kc // NT == qsb
                if diag:
                    nc.tensor.matmul(
                        p_s[:, :], ident_r[:, :], maskt[:, kc % NT, :],
                        start=True, stop=False,
                    )
                nc.tensor.matmul(
                    p_s[:, :],
                    kt_l[hp : hp + DH, toff + kc * 128 : toff + (kc + 1) * 128],
                    qt_l[hp : hp + DH, toff + qsb * T : toff + (qsb + 1) * T],
                    start=not diag, stop=True,
                )
                pt = pts.tile([128, T], BF16, tag="pt", name="pt")
                nc.scalar.activation(pt[:, :], p_s[:, :], EXP, scale=0.125)
                nc.tensor.matmul(
                    p_o[:, :],
                    va_l[:, bb * GROUP * NT + kc, lh * (DH + 1) : (lh + 1) * (DH + 1)],
                    pt[:, :],
                    start=(kc == 0), stop=(kc == nkc - 1),
                )

        def normalize(bb, lh, qsb, p_o):
            rc = otst.tile([65, T], F32R, tag="rc", name="rc")
            with nc.allow_low_precision(reason="fp32r feeds the bcast matmul"):
                nc.vector.reciprocal(rc[DH : DH + 1, :], p_o[DH : DH + 1, :])
            p_nb = ps.tile([DH, T], F32, tag="pc", name="p_nb")
            nc.tensor.matmul(
                p_nb[:, :], ones65[DH : DH + 1, 0:DH], rc[DH : DH + 1, :],
                start=True, stop=True,
            )
            ot_s = otst.tile([65, T], F32, tag="ot", name="ot_s")
            nc.vector.tensor_copy(ot_s[:, :], p_o[:, :])
            nc.vector.tensor_mul(ot_s[0:DH, :], ot_s[0:DH, :], p_nb[:, :])
            nc.sync.dma_start(
                out=a2_in.ap()[bb * GROUP + qsb, lh * DH : (lh + 1) * DH, :],
                in_=ot_s[0:DH, :],
            )

        for bb in range(B):
            for qsb in range(GROUP):
                # head pair interleaved: lhsT base partitions 0 and 64 ->
                # the PE runs both K=64 score matmuls in separate row groups
                p_o0 = ps.tile([DH + 1, T], F32, tag="pb", name="p_o0")
                p_o1 = ps.tile([DH + 1, T], F32, tag="pb", name="p_o1")
                attend(bb, 0, qsb, p_o0)
                attend(bb, 1, qsb, p_o1)
                normalize(bb, 0, qsb, p_o0)
                normalize(bb, 1, qsb, p_o1)

        ot_cm.__exit__(None, None, None)
        pts_cm.__exit__(None, None, None)
        att_cm.__exit__(None, None, None)

        # ---- A2A #2: attention rows back to token-sharding ----
        nc.gpsimd.collective_compute(
            "AllToAll",
            mybir.AluOpType.bypass,
            replica_groups=RG,
            ins=[a2_in.ap().opt()],
            outs=[a2_out.ap().opt()],
        )

        # ---- phase 4: out.T = W_O @ attn.T ----
        np4_cm = tc.tile_pool(name="np4", bufs=2)
        np4 = np4_cm.__enter__()
        attn_t = np4.tile([128, ND, T], F32R, bufs=1)
        a2_out_r = a2_out.ap().bitcast(F32R)
        for j in range(NCORES):
            nc.sync.dma_start(
                out=attn_t[:, j, :], in_=a2_out_r[j, :, :]
            )
        for c2 in range(ND):
            p_out = ps.tile([128, T], F32, tag="pa", bufs=3, name="p_out")
            for c1 in range(ND):
                nc.tensor.matmul(
                    p_out[:, :],
                    wot_s[:, c1, c2 * 128 : (c2 + 1) * 128],
                    attn_t[:, c1, :],
                    start=(c1 == 0), stop=(c1 == ND - 1),
                )
            oo = np4.tile([128, T], F32, tag="oo", name="oo")
            nc.vector.tensor_copy(oo[:, :], p_out[:, :])
            nc.sync.dma_start(
                out=out_e.ap()[c2 * 128 : (c2 + 1) * 128, :], in_=oo[:, :]
            )
        np4_cm.__exit__(None, None, None)
        wot_cm.__exit__(None, None, None)

    return nc


_NC_CACHE = None


def _get_nc():
    global _NC_CACHE
    if _NC_CACHE is None:
        nc = build_kernel()
        if not nc.is_finalized():
            nc.finalize()  # Bacc: runs wait legalization + reg alloc
        _NC_CACHE = nc
    return _NC_CACHE


def _make_maskdiag():
    """Diagonal-block additive masks: maskd[o, p, j] masks k-chunk offset o
    within the query 512-block (valid iff o*128 + p <= j)."""
    o = np.arange(NT)[:, None, None] * 128
    p = np.arange(128)[None, :, None]
    j = np.arange(T)[None, None, :]
    return np.where(o + p <= j, 0.0, MASK_NEG).astype(np.float32)


def make_in_maps(inputs):
    x = np.ascontiguousarray(inputs["x"], dtype=np.float32)
    f_qk = np.ascontiguousarray(inputs["f_qk"], dtype=np.float32)
    f_v = np.ascontiguousarray(inputs["f_v"], dtype=np.float32)
    r_qk = np.ascontiguousarray(inputs["r_qk"], dtype=np.float32)
    r_v = np.ascontiguousarray(inputs["r_v"], dtype=np.float32)
    w_ot = np.ascontiguousarray(inputs["W_O"].T, dtype=np.float32)
    maskd = _make_maskdiag()

    in_maps = []
    for c in range(NCORES):
        b, g = c // GROUP, c % GROUP
        sl = slice(g * T, (g + 1) * T)
        w1 = np.stack(
            [
                inputs["fqk_weights_Q"][b, sl].T,
                inputs["fqk_weights_K"][b, sl].T,
                inputs["fv_weights"][b, sl].T,
            ]
        ).astype(np.float32)
        w2 = np.stack(
            [
                inputs["rqk_weights_Q"][b, sl].T,
                inputs["rqk_weights_K"][b, sl].T,
                inputs["rv_weights"][b, sl].T,
            ]
        ).astype(np.float32)
        in_maps.append(
            {
                "xT": np.ascontiguousarray(x[b, sl].T),
                "w1": np.ascontiguousarray(w1),
                "w2": np.ascontiguousarray(w2),
                "f_qk": f_qk,
                "f_v": f_v,
                "r_qk": r_qk,
                "r_v": r_v,
                "w_ot": w_ot,
                "maskd": maskd,
            }
        )
    return in_maps


def assemble_output(results):
    out = np.zeros((B, S, D), dtype=np.float32)
    for c in range(NCORES):
        b, g = c // GROUP, c % GROUP
        out[b, g * T : (g + 1) * T, :] = results[c]["out"].T
    return out


def kernel(**inputs):
    nc = _get_nc()
    in_maps = make_in_maps(inputs)
    res = run_bass_kernel_spmd(nc, in_maps, core_ids=list(range(NCORES)))
    return assemble_output(res.results)


if __name__ == "__main__":
    nc = build_kernel()
    nc.finalize()
    print("built ok")
